# revision 32
# baseline (speedup 1.0000x reference)
"""Mixture-of-Softmaxes with shared embedding — 8-core Trainium2 Bass kernel.

Strategy (tensor-parallel on the vocab output head, per the sharding hint):
  - Vocab dim V is sharded across the 8 cores (Vp = 6283 rows each, zero-padded
    from 50257 to 50264; the 7 pad rows contribute exactly exp(0)*XS each
    to the softmax denominator and are corrected by a constant subtraction).
  - The expert transforms (10 experts x 2560x2560) are sharded as 200
    (expert, d-block) jobs, 25 per core, followed by an AllGather of the
    fp8 expert_hidden^T (6.5 MB, in two pieces of 13/12 jobs per core —
    fewer pieces minimize the serialized per-collective latency on the CC
    channel; the first piece also eats the first-collective warmup).
  - Pass-1 lhsT reads DIRECTLY from the per-(piece,core) AllGather stage
    tiles (no SBUF->SBUF regroup copies); the 4 k-pairs that straddle a
    piece-0 core boundary are patched into tiny fixup tiles.
  - The big vocab matmul runs in fp8 (e4m3) with DoubleRow perf mode
    (K=256 per instruction): embedding is scaled x64 on the host, the
    expert weights x16 (so expert_hidden lands x16 in fp8); the per-token
    RMS factor r_s and the 1/1024 descale are folded into the per-partition
    scale of the fused exp (pass-1 partitions are tokens), with a -3*ln2
    bias so exp values stay below TRN fp8-e4m3's +-240 ceiling.
  - exp values X are stored in fp8 (spills and SBUF-kept chunks), halving
    the spill bandwidth; the mixture reads fp8 directly (DVE STT and
    bf16-diag x fp8 PE matmuls both accept it).
  - Softmax over the full vocab needs a cross-core reduction: since the
    logits are O(1), exp() cannot overflow in fp32, so the max-shift is
    unnecessary and the reduction collapses to one AllReduce-ADD of the
    (128,10) sum-of-exp stats per token-half.
  - DMA rings: expert-weight streams alternate sync/scalar; embT chunks and
    spill reloads ride sync; spills, h32, stage loads and outputs ride
    scalar; Z-stats ride GpSimd SWDGE.
  - Half 0's mixture + log + store run inside half 1's pass-1 window (DVE
    mix, ACT ln, scalar-ring store all have slack there); half 1's mixture
    splits PE (9 chunks) / DVE (4 kept chunks) right after the Z AllReduce.

kernel(**inputs) takes the full unsharded inputs and returns the full
(1, 256, 50257) float32 logits.
"""
import sys

for _p in ("/opt/trn_rl_repo",):
    if _p not in sys.path:
        sys.path.append(_p)

import numpy as np
import ml_dtypes

import concourse.bacc as bacc
import concourse.mybir as mybir
import concourse.tile as tile
from concourse.bass_utils import run_bass_kernel_spmd
from concourse.masks import make_identity

BF16 = ml_dtypes.bfloat16
FP8 = ml_dtypes.float8_e4m3

NCORES = 8
S = 256          # tokens (B*S)
H = 2560         # hidden
E = 10           # experts
V = 50257        # vocab
KB = H // 128    # 20 k-blocks
NJOBS = E * KB   # 200 (expert, d-block) jobs
JPC = NJOBS // NCORES  # 25 jobs per core
AGP = (13, 12)   # jobs per core in each AllGather piece
AGOFF = (0, 13)  # slot offsets of the pieces
VP = 6283        # per-core vocab slice (8*6283 = 50264)
NPAD = NCORES * VP - V  # 7 zero-pad vocab rows (on the last core)
CHUNK = 512
NCH = (VP + CHUNK - 1) // CHUNK  # 13
CHUNKS = [(i * CHUNK, min(CHUNK, VP - i * CHUNK)) for i in range(NCH)]
NKEEP = 6        # trailing half-1 chunks kept in SBUF (skip spill roundtrip)
NPE = NCH - 4    # chunks mixed on PE in the tail (rest on DVE)
EPS_NORM = 1e-05
EPS_LOG = 1e-10
EMB_SCALE = 64.0
EH_SCALE = 16.0  # folded into wjobs on the host
INV_SCALE = 1.0 / (EMB_SCALE * EH_SCALE)
# exp bias: X = exp(l)*XS.  XS=1/8 keeps the hottest logit (~6.7 + fp8
# noise) below TRN-e4m3's 240 ceiling (threshold ln(240/XS)=7.56), while
# flushed tails (p < 2^-9/(XS*Z) ~ 4e-7) contribute ~4e-10 to a mixture
# whose smallest value is ~9e-7 — negligible.
XS = 0.125
XBIAS = -2.0794415416798357  # ln(XS) = -3*ln2

# piece-0 k-pairs that straddle a core boundary (global even job ids)
FIXUPS = (12, 38, 64, 90)

_nc_cache = None


def _job_of(c, slot):
    """Global job id (= e*KB + k) held by core c at slot (0..JPC-1)."""
    for p, (n, off) in enumerate(zip(AGP, AGOFF)):
        if slot < off + n:
            base = NCORES * sum(AGP[:p])
            return base + c * n + (slot - off)
    raise ValueError(slot)


def _loc_of(g):
    """(piece, core, slot-in-block) of global job id g."""
    for p in range(len(AGP)):
        base = NCORES * sum(AGP[:p])
        if g < base + NCORES * AGP[p]:
            idx = g - base
            return p, idx // AGP[p], idx % AGP[p]
    raise ValueError(g)


def build_kernel():
    global _nc_cache
    if _nc_cache is not None:
        return _nc_cache
    f32 = mybir.dt.float32
    bf = mybir.dt.bfloat16
    f8 = mybir.dt.float8e4
    u32 = mybir.dt.uint32
    nc = bacc.Bacc("TRN2", target_bir_lowering=False, debug=False, num_devices=NCORES)

    h32 = nc.declare_dram_parameter("h32", [2, 128, H], f32, isOutput=False)
    hT = nc.declare_dram_parameter("hT", [128, KB, S], bf, isOutput=False)
    gw = nc.declare_dram_parameter("gw", [128, KB, E], bf, isOutput=False)
    wj = nc.declare_dram_parameter("wjobs", [JPC, 128, KB, 128], bf, isOutput=False)
    embT = nc.declare_dram_parameter("embT", [NCH, 128, KB, CHUNK], f8, isOutput=False)
    out = nc.declare_dram_parameter("out", [S, VP], f32, isOutput=True)

    # partition-major AllGather buffers: per-rank (128, jobs, S)
    ehl = [
        nc.dram_tensor(f"eh_local{p}", [128, AGP[p], S], f8)
        for p in range(len(AGP))
    ]
    eha = [
        nc.dram_tensor(
            f"eh_all{p}", [NCORES * 128, AGP[p], S], f8, addr_space="Shared"
        )
        for p in range(len(AGP))
    ]
    zl = [nc.dram_tensor(f"zl{sh}", [128, E], f32) for sh in range(2)]
    za = [
        nc.dram_tensor(f"za{sh}", [128, E], f32, addr_space="Shared")
        for sh in range(2)
    ]
    xsp = nc.dram_tensor("xspill", [NCH, 2, 128, E, CHUNK], f8)

    rg = [list(range(NCORES))]

    with tile.TileContext(nc) as tc:
        with (
            tc.tile_pool(name="et", bufs=5) as etp,          # 10KB slots
            tc.tile_pool(name="stream", bufs=5) as strm,     # 5.1KB slots
            tc.tile_pool(name="xs", bufs=NKEEP + 1) as xsp_pool,  # 5.1KB slots
            tc.tile_pool(name="stg", bufs=1) as stgp,        # persistent stages
            tc.tile_pool(name="acc", bufs=4) as accp,
            tc.tile_pool(name="ot", bufs=3) as otp,
            tc.tile_pool(name="persist", bufs=1) as per,
            tc.tile_pool(name="psmall", bufs=3, space="PSUM") as psS,
            tc.tile_pool(name="psbig", bufs=5, space="PSUM") as psC,
        ):
            # gpsimd-engine constants FIRST: the collectives below occupy the
            # gpsimd queue with long waits, so anything queued after them
            # would only materialize ~100us in.
            epsl = per.tile([128, 1], f32, tag="epsl")
            nc.gpsimd.memset(epsl, EPS_LOG)
            epsn = per.tile([128, 1], f32, tag="epsn")
            nc.gpsimd.memset(epsn, EPS_NORM)
            xbias = per.tile([128, 1], f32, tag="xbias")
            nc.gpsimd.memset(xbias, XBIAS)
            # identity for the diag-mix weights of the tail
            ident = per.tile([128, 128], bf, tag="ident")
            make_identity(nc, ident)

            # raw h^T (bf16) — gate + expert matmuls use it un-normalized.
            # One DMA, first on sync: each extra small DMA costs a serial
            # ~2-3us ring round-trip, so splitting it is a net loss.
            hTr = per.tile([128, KB, S], bf, tag="hTr")
            nc.sync.dma_start(out=hTr, in_=hT[:])
            gw3 = per.tile([128, KB, E], bf, tag="gw3")
            nc.sync.dma_start(out=gw3, in_=gw[:])

            # ---- expert transform shard: 25 (e, dblk) jobs ----
            # wjobs carry x16 (fp8 headroom) so the drain is a plain copy
            elbufs = [
                per.tile([128, AGP[p], S], f8, tag=f"elbuf{p}", name=f"elbuf{p}")
                for p in range(len(AGP))
            ]
            # persistent AllGather stage tiles: pass-1 lhsT reads these
            stg = [
                [
                    stgp.tile([128, AGP[p], S], f8, tag=f"stg{p}_{c}",
                              name=f"stg{p}_{c}")
                    for c in range(NCORES)
                ]
                for p in range(len(AGP))
            ]
            h32t = [None, None]
            et_pre = []
            for j in range(JPC):
                wjt = strm.tile([128, KB, 128], bf, tag="stream")
                eng = nc.sync if j % 2 == 0 else nc.scalar
                eng.dma_start(out=wjt, in_=wj[j])
                if j in (4, 6, 8, 10):
                    # h32 for the RMS factors, spread as four half-loads so
                    # neither the scalar ring nor the DVE bn_stats burst
                    # displaces the expert-weight stream
                    qi = (j - 4) // 2
                    sh_, hhalf = divmod(qi, 2)
                    if hhalf == 0:
                        h32t[sh_] = etp.tile([128, H], f32, tag="et",
                                             name=f"h32t{sh_}")
                    nc.scalar.dma_start(
                        out=h32t[sh_][:, hhalf * (H // 2) : (hhalf + 1) * (H // 2)],
                        in_=h32[sh_, :, hhalf * (H // 2) : (hhalf + 1) * (H // 2)],
                    )
                if j in (2, 4, 8):
                    # prefetch half-0's first embedding chunks (scalar ring —
                    # sync must keep the expert-weight stream fed, and the
                    # stage loads blocking on the AllGathers come later)
                    et3 = etp.tile([128, KB, CHUNK], f8, tag="et")
                    nc.scalar.dma_start(out=et3, in_=embT[len(et_pre)])
                    et_pre.append(et3)
                bps = psS.tile([128, S], f32, tag="ps_small")
                for k in range(KB):
                    nc.tensor.matmul(
                        bps, wjt[:, k, :], hTr[:, k, :],
                        start=(k == 0), stop=(k == KB - 1),
                    )
                for p in range(len(AGP)):
                    if AGOFF[p] <= j < AGOFF[p] + AGP[p]:
                        nc.vector.tensor_copy(
                            elbufs[p][:, j - AGOFF[p], :], bps
                        )
                        if j == AGOFF[p] + AGP[p] - 1:
                            nc.scalar.dma_start(out=ehl[p][:], in_=elbufs[p])
                            nc.gpsimd.collective_compute(
                                "AllGather", mybir.AluOpType.bypass,
                                replica_groups=rg,
                                ins=[ehl[p][:]], outs=[eha[p][:]],
                            )

            # ---- gate logits: fills the PE window between the last job and
            # the AllGather-gated start of pass 1 ----
            glog = []
            for sh in range(2):
                gps = psS.tile([128, E], f32, tag="ps_small")
                for k in range(KB):
                    nc.tensor.matmul(
                        gps,
                        hTr[:, k, sh * 128 : (sh + 1) * 128],
                        gw3[:, k, :],
                        start=(k == 0),
                        stop=(k == KB - 1),
                    )
                gl = per.tile([128, E], f32, tag=f"glog{sh}")
                nc.vector.tensor_copy(gl, gps)
                glog.append(gl)

            # AllGather stage loads, emitted AFTER the whole job loop: they
            # block on the collectives, and anything queued behind them on
            # the same ring would head-block (v3 lost ~30us to wjt loads
            # stuck behind stage piece 0).  Post-loop, the only sync-ring
            # work behind them (pass-1 embT chunks 3+, spills) isn't needed
            # until well after the AllGathers land.
            for p in range(len(AGP)):
                for c in range(NCORES):
                    nc.sync.dma_start(
                        out=stg[p][c],
                        in_=eha[p][c * 128 : (c + 1) * 128, :, :],
                    )

            fx = {}

            def lhsT_of(e, k2, sh):
                g0 = e * KB + 2 * k2
                if g0 in fx:
                    return fx[g0][:, :, sh * 128 : (sh + 1) * 128]
                p, c, sl = _loc_of(g0)
                return stg[p][c][:, sl : sl + 2, sh * 128 : (sh + 1) * 128]

            # ---- per-token RMS factors r_s = 1/sqrt(mean(h^2)+eps) ----
            NSG = H // nc.vector.BN_STATS_FMAX
            r = []
            for sh in range(2):
                ht = h32t[sh]
                stats = per.tile(
                    [128, NSG, nc.vector.BN_STATS_DIM], f32, tag=f"st{sh}"
                )
                for sg in range(NSG):
                    nc.vector.bn_stats(
                        out=stats[:, sg, :],
                        in_=ht[
                            :,
                            sg * nc.vector.BN_STATS_FMAX : (sg + 1)
                            * nc.vector.BN_STATS_FMAX,
                        ],
                    )
                mv = per.tile([128, nc.vector.BN_AGGR_DIM], f32, tag=f"mv{sh}")
                nc.vector.bn_aggr(out=mv, in_=stats)
                msq = per.tile([128, 1], f32, tag=f"msq{sh}")
                nc.vector.scalar_tensor_tensor(
                    out=msq, in0=mv[:, 0:1], scalar=mv[:, 0:1], in1=mv[:, 1:2],
                    op0=mybir.AluOpType.mult, op1=mybir.AluOpType.add,
                )
                rsd = per.tile([128, 1], f32, tag=f"rsd{sh}")
                nc.scalar.activation(
                    out=rsd, in_=msq, func=mybir.ActivationFunctionType.Sqrt,
                    bias=epsn[:, 0:1],
                )
                rt = per.tile([128, 1], f32, tag=f"r{sh}")
                nc.vector.reciprocal(rt, rsd)
                r.append(rt)

            # exp scale for pass 1: r_s * 1/1024 (per-partition = per-token)
            rinv = []
            for sh in range(2):
                ri = per.tile([128, 1], f32, tag=f"rinv{sh}")
                nc.vector.tensor_scalar_mul(ri, r[sh], INV_SCALE)
                rinv.append(ri)

            # ---- gate softmax g (no max shift; logits are O(1)) ----
            g = []
            for sh in range(2):
                ge = per.tile([128, E], f32, tag=f"ge{sh}")
                gsum = per.tile([128, 1], f32, tag=f"gsum{sh}")
                nc.scalar.activation(
                    out=ge, in_=glog[sh], func=mybir.ActivationFunctionType.Exp,
                    scale=r[sh][:, 0:1], accum_out=gsum[:, 0:1],
                )
                grc = per.tile([128, 1], f32, tag=f"grc{sh}")
                nc.vector.reciprocal(grc, gsum)
                gt = per.tile([128, E], f32, tag=f"g{sh}")
                nc.vector.tensor_scalar_mul(gt, ge, grc[:, 0:1])
                g.append(gt)

            # fixup tiles for the piece-0 k-pairs split across core blocks.
            # Emitted AFTER the RMS/gate work: they wait on AllGather piece 0,
            # and the vector queue is FIFO — earlier emission would stall
            # bn_stats (and so rinv, needed by pass-1's first exp) behind it.
            for g0 in FIXUPS:
                t = per.tile([128, 2, S], f8, tag=f"fx{g0}", name=f"fx{g0}")
                for half, gj in enumerate((g0, g0 + 1)):
                    p, c, sl = _loc_of(gj)
                    nc.vector.tensor_copy(
                        out=t[:, half, :].bitcast(u32),
                        in_=stg[p][c][:, sl, :].bitcast(u32),
                    )
                fx[g0] = t

            # ---- pass 1, both halves back to back (PE never leaves the
            # big-matmul stream; all cross-pass work rides other queues).
            def pass1(sh, zacc, es=0, ee=E, tick=None, pre=(), keep=False):
                # one sweep of experts [es, ee) over all vocab chunks.
                # Half 0 runs as two 5-expert sweeps (re-reading embT) so the
                # first sweep starts right after AllGather piece 0 instead of
                # stalling on piece 1 mid-chunk.
                kept = {}
                ne = ee - es
                for ci, (v0, vn) in enumerate(CHUNKS):
                    if ci < len(pre):
                        et3 = pre[ci]
                    else:
                        et3 = etp.tile([128, KB, CHUNK], f8, tag="et")
                        nc.sync.dma_start(out=et3, in_=embT[ci])
                    if ne == E:
                        xs3 = xsp_pool.tile([128, ne, CHUNK], f8, tag="xs")
                    else:
                        xs3 = xsp_pool.tile([128, ne, CHUNK], f8, tag="xsh",
                                            bufs=4)
                    for e in range(es, ee):
                        cps = psC.tile([128, CHUNK], f32, tag="psC")
                        for k2 in range(KB // 2):
                            nc.tensor.matmul(
                                cps[:, :vn],
                                lhsT_of(e, k2, sh),
                                et3[:, 2 * k2 : 2 * k2 + 2, :vn],
                                start=(k2 == 0),
                                stop=(k2 == KB // 2 - 1),
                                perf_mode=mybir.MatmulPerfMode.DoubleRow,
                            )
                        zc = per.tile([128, 1], f32, tag="zc", bufs=24)
                        nc.scalar.activation(
                            out=xs3[:, e - es, :vn], in_=cps[:, :vn],
                            func=mybir.ActivationFunctionType.Exp,
                            scale=rinv[sh][:, 0:1],
                            bias=xbias[:, 0:1],
                            accum_out=zc[:, 0:1],
                        )
                        if ci == 0:
                            nc.gpsimd.tensor_copy(zacc[:, e : e + 1], zc)
                        else:
                            nc.gpsimd.tensor_add(
                                zacc[:, e : e + 1], zacc[:, e : e + 1], zc
                            )
                    if keep and ci >= NCH - NKEEP:
                        kept[ci] = xs3
                    else:
                        nc.sync.dma_start(
                            out=xsp[ci, sh, :, es:ee], in_=xs3
                        )
                    if tick is not None:
                        tick(ci)
                return kept

            def z_allreduce(sh, zacc):
                # store + collective + load all on gpsimd; the deep zc pool
                # absorbs the collective wait so pass-1 exps never stall.
                nc.gpsimd.dma_start(out=zl[sh][:], in_=zacc)
                nc.gpsimd.collective_compute(
                    "AllReduce", mybir.AluOpType.add, replica_groups=rg,
                    ins=[zl[sh][:]], outs=[za[sh][:]],
                )
                zs = per.tile([128, E], f32, tag=f"zs{sh}")
                nc.gpsimd.dma_start(out=zs, in_=za[sh][:])
                # pad vocab rows each contribute exp(0 - ln2) = 0.5
                nc.vector.tensor_scalar_add(zs, zs, float(-NPAD * XS))
                zrc = per.tile([128, E], f32, tag=f"zrc{sh}")
                nc.vector.reciprocal(zrc, zs)
                Rt = per.tile([128, E], f32, tag=f"R{sh}")
                nc.vector.tensor_mul(Rt, g[sh], zrc)
                return Rt

            def mix_dve(xt3, Rt, vn):
                accd = accp.tile([128, CHUNK], bf, tag="accd")
                nc.vector.tensor_scalar_mul(
                    accd[:, :vn], xt3[:, 0, :vn], Rt[:, 0:1]
                )
                for e in range(1, E):
                    nc.vector.scalar_tensor_tensor(
                        out=accd[:, :vn],
                        in0=xt3[:, e, :vn],
                        scalar=Rt[:, e : e + 1],
                        in1=accd[:, :vn],
                        op0=mybir.AluOpType.mult,
                        op1=mybir.AluOpType.add,
                    )
                return accd

            def ln_out(src, sh, v0, vn):
                ot = otp.tile([128, CHUNK], f32, tag="ot")
                nc.scalar.activation(
                    out=ot[:, :vn], in_=src[:, :vn],
                    func=mybir.ActivationFunctionType.Ln,
                    bias=epsl[:, 0:1],
                )
                nc.scalar.dma_start(
                    out=out[sh * 128 : (sh + 1) * 128, v0 : v0 + vn],
                    in_=ot[:, :vn],
                )

            zacc0 = per.tile([128, E], f32, tag="zacc0")
            pass1(0, zacc0, 0, 5, pre=et_pre)
            pass1(0, zacc0, 5, 10)
            Rt0 = z_allreduce(0, zacc0)  # overlaps half 1's stream

            # ---- pass 2, half 0 is fully interleaved into half 1's pass-1
            # emission: one spill reload (sync ring) + one DVE mix + ln +
            # store per chunk, all in engines with slack during half 1.
            xrl0 = [
                strm.tile([128, E, CHUNK], f8, tag="stream", name=f"x0_{ci}")
                for ci in range(NCH)
            ]

            def tick(ci1):
                ci = ci1 - 1
                if 0 <= ci < NCH:
                    nc.sync.dma_start(out=xrl0[ci], in_=xsp[ci, 0])
                ci = ci1 - 2
                if 0 <= ci < NCH:
                    acc = mix_dve(xrl0[ci], Rt0, CHUNKS[ci][1])
                    ln_out(acc, 0, *CHUNKS[ci])

            zacc1 = per.tile([128, E], f32, tag="zacc1")
            kept1 = pass1(1, zacc1, tick=tick, keep=True)

            # Z AllReduce for half 1 goes out IMMEDIATELY after the last exp;
            # the remaining half-0 mixes and the spill reloads fill its wait.
            nc.gpsimd.dma_start(out=zl[1][:], in_=zacc1)
            nc.gpsimd.collective_compute(
                "AllReduce", mybir.AluOpType.add, replica_groups=rg,
                ins=[zl[1][:]], outs=[za[1][:]],
            )

            nc.sync.dma_start(out=xrl0[NCH - 1], in_=xsp[NCH - 1, 0])
            for ci in (NCH - 2, NCH - 1):
                acc = mix_dve(xrl0[ci], Rt0, CHUNKS[ci][1])
                ln_out(acc, 0, *CHUNKS[ci])

            # ---- tail reloads for half 1's spilled chunks (sync ring,
            # self-paced by pool slots; overlaps the Z AllReduce).
            xrl1 = {}
            for ci in range(NCH):
                if ci in kept1:
                    xrl1[ci] = kept1[ci]
                else:
                    xt3 = strm.tile([128, E, CHUNK], f8, tag="stream",
                                    name=f"x1_{ci}")
                    xrl1[ci] = xt3
                    nc.sync.dma_start(out=xt3, in_=xsp[ci, 1])

            zs = per.tile([128, E], f32, tag="zs1")
            nc.gpsimd.dma_start(out=zs, in_=za[1][:])
            nc.vector.tensor_scalar_add(zs, zs, float(-NPAD * XS))
            zrc = per.tile([128, E], f32, tag="zrc1")
            nc.vector.reciprocal(zrc, zs)
            Rt1 = per.tile([128, E], f32, tag="R1")
            nc.vector.tensor_mul(Rt1, g[1], zrc)

            # diag(R_e) weights for the PE-side mix (vector: fast, and right
            # after the Rt1 chain so the first LDWEIGHTS isn't kept waiting)
            dti = []
            for e in range(E):
                dt_ = per.tile([128, 128], bf, tag=f"diag{e}")
                nc.vector.tensor_scalar_mul(dt_, ident, Rt1[:, e : e + 1])
                dti.append(dt_)

            # Tail mixes: PE takes NPE chunks (kept ones first — instantly
            # available — then reloads in arrival order), DVE takes the last
            # kept chunks.  ALL PE lns are emitted before the DVE ones: the
            # ACT queue is FIFO, and a PE chunk's psC slot can't recycle
            # until its ln runs — interleaving them behind slow DVE mixes
            # would stall the PE.
            pe_order = [ci for ci in range(NPE) if ci in kept1] + [
                ci for ci in range(NPE) if ci not in kept1
            ]
            for ci in pe_order:
                v0, vn = CHUNKS[ci]
                mps = psC.tile([128, CHUNK], f32, tag="psC")
                for e in range(E):
                    nc.tensor.matmul(
                        mps[:, :vn], dti[e], xrl1[ci][:, e, :vn],
                        start=(e == 0), stop=(e == E - 1),
                    )
                ln_out(mps, 1, v0, vn)
            for ci in range(NPE, NCH):
                accd = mix_dve(xrl1[ci], Rt1, CHUNKS[ci][1])
                ln_out(accd, 1, *CHUNKS[ci])

    nc.compile()
    _nc_cache = nc
    return nc


def prepare_in_maps(inputs):
    h = np.asarray(inputs["hidden_states"], np.float32).reshape(S, H)
    emb = np.asarray(inputs["embedding_matrix"], np.float32)
    ns = np.asarray(inputs["norm_scale"], np.float32)
    W = np.asarray(inputs["expert_weights"], np.float32)
    G = np.asarray(inputs["gate_weight"], np.float32)

    h32 = np.ascontiguousarray(h.reshape(2, 128, H))
    # hT[p, k, s] = h[s, k*128+p]
    hTb = np.ascontiguousarray(h.reshape(S, KB, 128).transpose(2, 1, 0)).astype(BF16)
    gwb = np.ascontiguousarray(
        (G * ns[:, None]).reshape(KB, 128, E).transpose(1, 0, 2)
    ).astype(BF16)

    Wn = W * (ns[None, :, None] * EH_SCALE)
    # wjobs_all[j = e*KB + dblk, p, k, d] = Wn[e, k*128+p, dblk*128+d]
    Wr = Wn.reshape(E, KB, 128, KB, 128)
    wjobs_all = np.ascontiguousarray(
        Wr.transpose(0, 3, 2, 1, 4).reshape(NJOBS, 128, KB, 128)
    ).astype(BF16)

    VPAD = NCH * CHUNK  # 6656 (layout padding only; compute uses VP)
    embp = np.zeros((NCORES * VP + (VPAD - VP), H), np.float32)
    embp[:V] = emb

    job_order = [
        [(_job_of(c, slot)) for slot in range(JPC)] for c in range(NCORES)
    ]

    in_maps = []
    for c in range(NCORES):
        esl = embp[c * VP : c * VP + VPAD]  # (VPAD, H) with layout pad tail
        # embT_c[ci, p, k, v] = esl[ci*CHUNK+v, k*128+p] * EMB_SCALE
        embT_c = (
            np.ascontiguousarray(
                esl.reshape(NCH, CHUNK, KB, 128).transpose(0, 3, 2, 1)
            )
            * EMB_SCALE
        ).astype(FP8)
        in_maps.append(
            {
                "h32": h32,
                "hT": hTb,
                "gw": gwb,
                "wjobs": np.ascontiguousarray(wjobs_all[job_order[c]]),
                "embT": embT_c,
            }
        )
    return in_maps


def assemble_output(results):
    full = np.concatenate([results[c]["out"] for c in range(NCORES)], axis=1)
    return np.ascontiguousarray(full[:, :V].reshape(1, S, V).astype(np.float32))


def kernel(**inputs):
    nc = build_kernel()
    in_maps = prepare_in_maps(inputs)
    res = run_bass_kernel_spmd(nc, in_maps, list(range(NCORES)))
    return assemble_output(res.results)
